# revision 6
# baseline (speedup 1.0000x reference)
"""Bass/Trainium2 kernel for nn_GAT_25082609009415.

GAT: g = x[46,131072] @ W1[131072,2048] -> 8-head masked attention ->
ELU -> h @ W2[2048,64] -> 1-head attention -> mean -> MLP(46->12->1) -> sigmoid.

Strategy (8 NeuronCores), v2:
  * K-shard the dominant GEMM: core c streams W1[16384c:16384(c+1), :]
    in fp8e4 (host-scaled by 2^12; x scaled by 2^5) -- 33.5 MB/core,
    4x fewer HBM bytes than the fp32 baseline.  x arrives
    host-pretransposed as [128, kt, 46] fp8, so no on-device transposes.
  * W1 is laid out in G=4 column groups of 512; each group's partial
    g accumulates in its own PSUM bank and is AllReduced while the next
    group's GEMM streams -- 3 of 4 collectives are hidden.
  * e_src/e_dst are linear in g, so their per-core partials are computed
    from the local partial g and ride along in the last AllReduce.
  * Tail attention avoids gT entirely: e_src/e_dst via vector
    mult+reduce, e_dst broadcast via tiny PE outer-product matmuls,
    h1 computed directly transposed (h1T) so layer-2 needs no
    transposes.  Single activation table (Exp), preloaded during the
    GEMM; sigmoid is computed via exp+reciprocal.
"""
import numpy as np
import ml_dtypes

import concourse.bass as bass
import concourse.bacc as bacc
import concourse.tile as tile
from concourse import mybir
from concourse.bass_utils import run_bass_kernel_spmd

N = 46
KTOT = 131072
HID = 2048
HEADS = 8
F1 = HID // HEADS          # 256 features / head
OUTF = 64
NCORES = 8
KC = KTOT // NCORES        # 16384 contraction elems per core
KT = KC // 128             # 128 k-tiles per core
KT2 = HID // 128           # 16 k-tiles for layer-2 GEMM
G = 4                      # column groups for pipelined AllReduce
GW = HID // G              # 512 columns per group
HPG = HEADS // G           # heads per group
TPD = 32                   # k-tiles per W1 DMA chunk (16KB/partition)
NCH = KT // TPD            # chunks per group

W_SCALE = float(2 ** 12)   # keeps W1 (~+-0.0028) in fp8e4 normal range
X_SCALE = float(2 ** 5)    # keeps x (~N(0,1)) well under fp8e4 max 240
DESCALE = 1.0 / (W_SCALE * X_SCALE)

F32 = mybir.dt.float32
F16 = mybir.dt.float16
F8 = mybir.dt.float8e4
AX = mybir.AxisListType
OP = mybir.AluOpType
ACTF = mybir.ActivationFunctionType

NP_F8 = ml_dtypes.float8_e4m3


def build():
    nc = bacc.Bacc(
        "TRN2",
        target_bir_lowering=False,
        debug=False,
        enable_asserts=False,
        num_devices=NCORES,
    )
    xs = nc.dram_tensor("xs", [128, KT * N], F8, kind="ExternalInput")
    w1 = nc.dram_tensor("w1", [128, G * KT * GW], F8, kind="ExternalInput")
    w2r = nc.dram_tensor("w2r", [128, KT2 * OUTF], F16, kind="ExternalInput")
    adj01 = nc.dram_tensor("adj01", [N, N], F32, kind="ExternalInput")
    asrcf = nc.dram_tensor("asrcf", [1, HID], F16, kind="ExternalInput")
    adstf = nc.dram_tensor("adstf", [1, HID], F16, kind="ExternalInput")
    a2sf = nc.dram_tensor("a2sf", [1, OUTF], F16, kind="ExternalInput")
    a2df = nc.dram_tensor("a2df", [1, OUTF], F16, kind="ExternalInput")
    sel = nc.dram_tensor("sel", [HEADS, HEADS * N], F32, kind="ExternalInput")
    ones46 = nc.dram_tensor("ones46", [1, N], F32, kind="ExternalInput")
    mw1 = nc.dram_tensor("mw1", [N, 12], F32, kind="ExternalInput")
    mb1 = nc.dram_tensor("mb1", [1, 12], F32, kind="ExternalInput")
    mw2t = nc.dram_tensor("mw2t", [1, 12], F32, kind="ExternalInput")
    mb2 = nc.dram_tensor("mb2", [1, 1], F32, kind="ExternalInput")
    ident = nc.dram_tensor("ident", [N, N], F32, kind="ExternalInput")
    out = nc.dram_tensor("out", [1, 1], F32, kind="ExternalOutput")

    # width of the last collective: g columns + esrc[8] + edst[8]
    W3 = GW + 2 * HEADS

    with tile.TileContext(nc) as tc:
        with (
            tc.tile_pool(name="psT", bufs=2, space="PSUM") as psT,
            tc.tile_pool(name="const", bufs=1) as cst,
            tc.tile_pool(name="sbxT", bufs=1) as sbxT,
            tc.tile_pool(name="sbw1", bufs=5) as sbw1,
            tc.tile_pool(name="sbst", bufs=2) as sbst,
            tc.tile_pool(name="sbg", bufs=1) as sbg,
            tc.tile_pool(name="sbt", bufs=1) as sbt,
            tc.tile_pool(name="sbsm", bufs=1) as sbsm,
            tc.tile_pool(name="dram", bufs=1, space="DRAM") as dram,
        ):
            # ---- phase A: x slices (pretransposed on host) + first W1 chunk
            # first, so the GEMM can start ASAP; constants go to the vector
            # queue to keep sync/scalar clear for the W1 stream.
            xT = sbxT.tile([128, KT, N], F8, tag="xT")
            XCH = KT // 4
            nc.sync.dma_start(xT[:, 0:XCH, :], xs.ap()[:, 0:XCH * N])
            w1c0 = sbw1.tile([128, TPD, GW], F8, tag="w1")
            nc.scalar.dma_start(w1c0[:], w1.ap()[:, 0:TPD * GW])
            for c in range(1, 4):
                q = nc.sync if c % 2 == 0 else nc.scalar
                q.dma_start(
                    xT[:, XCH * c:XCH * (c + 1), :],
                    xs.ap()[:, XCH * N * c:XCH * N * (c + 1)],
                )

            ident_sb = cst.tile([N, N], F32, tag="ident")
            nc.gpsimd.dma_start(ident_sb[:], ident.ap())
            adj01_sb = cst.tile([N, N], F32, tag="adj01")
            nc.gpsimd.dma_start(adj01_sb[:], adj01.ap())
            sel_sb = cst.tile([HEADS, HEADS * N], F32, tag="sel")
            nc.gpsimd.dma_start(sel_sb[:], sel.ap())
            ones46_sb = cst.tile([1, N], F32, tag="ones46")
            nc.gpsimd.dma_start(ones46_sb[:], ones46.ap())
            w2_sb = cst.tile([128, KT2, OUTF], F16, tag="w2")
            nc.gpsimd.dma_start(w2_sb[:], w2r.ap())
            asrc1 = cst.tile([1, HID], F16, tag="asrc1")
            nc.gpsimd.dma_start(asrc1[:], asrcf.ap())
            adst1 = cst.tile([1, HID], F16, tag="adst1")
            nc.gpsimd.dma_start(adst1[:], adstf.ap())
            a2s1 = cst.tile([1, OUTF], F16, tag="a2s1")
            nc.gpsimd.dma_start(a2s1[:], a2sf.ap())
            a2d1 = cst.tile([1, OUTF], F16, tag="a2d1")
            nc.gpsimd.dma_start(a2d1[:], a2df.ap())
            mw1_sb = cst.tile([N, 12], F32, tag="mw1")
            nc.gpsimd.dma_start(mw1_sb[:], mw1.ap())
            mb1_sb = cst.tile([1, 12], F32, tag="mb1")
            nc.gpsimd.dma_start(mb1_sb[:], mb1.ap())
            mw2t_sb = cst.tile([1, 12], F32, tag="mw2t")
            nc.gpsimd.dma_start(mw2t_sb[:], mw2t.ap())
            mb2_sb = cst.tile([1, 1], F32, tag="mb2")
            nc.gpsimd.dma_start(mb2_sb[:], mb2.ap())

            # broadcast attention vectors across the 46 node partitions
            asrcb = cst.tile([N, HID], F16, tag="asrcb")
            nc.gpsimd.partition_broadcast(asrcb[:], asrc1[:])
            adstb = cst.tile([N, HID], F16, tag="adstb")
            nc.gpsimd.partition_broadcast(adstb[:], adst1[:])
            a2sb = cst.tile([N, OUTF], F16, tag="a2sb")
            nc.gpsimd.partition_broadcast(a2sb[:], a2s1[:])
            a2db = cst.tile([N, OUTF], F16, tag="a2db")
            nc.gpsimd.partition_broadcast(a2db[:], a2d1[:])

            # preload the Exp activation table off the critical path
            dummy = sbsm.tile([1, 1], F32, tag="dummy")
            nc.scalar.activation(dummy[:], mb2_sb[:], ACTF.Exp)

            # single staged partial g + esrc/edst partials, one AllReduce
            WA = HID + 2 * HEADS
            stage_all = sbt.tile([N, WA], F32, tag="stage_all")
            cc_in = dram.tile([N, WA], F32, tag="ccin")
            cc_out = dram.tile([N, WA], F32, tag="ccout")
            g_bf = sbg.tile([N, HID], F16, tag="gbf")
            gfull = sbt.tile([N, WA], F32, tag="gfull")

            # ---- phase B: fp8 GEMM in G column groups ----
            with tc.tile_pool(name="psG", bufs=2, space="PSUM") as psG:
                ch = 1
                for g in range(G):
                    gps = psG.tile([N, GW], F32, tag="gps")
                    for c2 in range(NCH):
                        if g == 0 and c2 == 0:
                            w1c = w1c0
                        else:
                            w1c = sbw1.tile([128, TPD, GW], F8, tag="w1")
                            off = (g * KT + c2 * TPD) * GW
                            q = nc.sync if ch % 2 == 0 else nc.scalar
                            q.dma_start(w1c[:], w1.ap()[:, off:off + TPD * GW])
                            ch += 1
                        for t in range(TPD):
                            k = c2 * TPD + t
                            nc.tensor.matmul(
                                gps[:],
                                xT[:, k, :],
                                w1c[:, t, :],
                                start=(k == 0),
                                stop=(k == KT - 1),
                            )
                    # descale partial g out of PSUM
                    st_ap = stage_all[:, GW * g:GW * (g + 1)]
                    nc.vector.tensor_scalar_mul(st_ap, gps[:], DESCALE)

                    # partial e_src/e_dst for this group's 2 heads
                    tmp = sbst.tile([N, GW], F32, tag="etmp")
                    nc.vector.tensor_mul(
                        tmp[:], st_ap, asrcb[:, GW * g:GW * (g + 1)]
                    )
                    nc.vector.tensor_reduce(
                        stage_all[:, HID + HPG * g:HID + HPG * (g + 1)],
                        tmp[:].rearrange("p (h f) -> p h f", h=HPG),
                        axis=AX.X,
                        op=OP.add,
                    )
                    nc.vector.tensor_mul(
                        tmp[:], st_ap, adstb[:, GW * g:GW * (g + 1)]
                    )
                    nc.vector.tensor_reduce(
                        stage_all[:, HID + HEADS + HPG * g:HID + HEADS + HPG * (g + 1)],
                        tmp[:].rearrange("p (h f) -> p h f", h=HPG),
                        axis=AX.X,
                        op=OP.add,
                    )

            nc.sync.dma_start(cc_in[:], stage_all[:])
            nc.gpsimd.collective_compute(
                "AllReduce",
                OP.add,
                replica_groups=[list(range(NCORES))],
                ins=[cc_in[:].opt()],
                outs=[cc_out[:].opt()],
            )
            nc.sync.dma_start(gfull[:], cc_out[:])
            nc.vector.tensor_copy(g_bf[:], gfull[:, 0:HID])

            esrc = gfull[:, HID:HID + HEADS]           # [46, 8] fp32
            edst = gfull[:, HID + HEADS:WA]            # [46, 8] fp32

            with (
                tc.tile_pool(name="psE", bufs=1, space="PSUM") as psE,
                tc.tile_pool(name="psH", bufs=1, space="PSUM") as psH,
                tc.tile_pool(name="psS", bufs=1, space="PSUM") as psS,
            ):
                # ---- phase C: layer-1 attention (8 heads) ----
                # edst[j,h] -> edstT[h,j] -> broadcast over i via sel matmuls
                edT_ps = psT.tile([HEADS, N], F32, tag="tp")
                nc.tensor.transpose(edT_ps[:], edst, ident_sb[:])
                edT = sbsm.tile([HEADS, N], F32, tag="edT")
                nc.vector.tensor_copy(edT[:], edT_ps[:])
                ebc_ps = psE.tile([N, HEADS, N], F32, tag="ebc")
                for h in range(HEADS):
                    nc.tensor.matmul(
                        ebc_ps[:, h, :],
                        sel_sb[:, N * h:N * (h + 1)],
                        edT[:],
                        start=True,
                        stop=True,
                    )
                # e = leaky_relu(esrc_i + edst_j, 0.2); u = exp(e) * adj
                e_sb = sbsm.tile([N, HEADS, N], F32, tag="e")
                nc.vector.tensor_add(
                    e_sb[:],
                    ebc_ps[:],
                    esrc.unsqueeze(2).broadcast_to([N, HEADS, N]),
                )
                t02 = sbsm.tile([N, HEADS, N], F32, tag="t02")
                nc.vector.tensor_scalar_mul(t02[:], e_sb[:], 0.2)
                nc.vector.tensor_max(e_sb[:], e_sb[:], t02[:])
                u_sb = sbsm.tile([N, HEADS, N], F32, tag="u")
                nc.scalar.activation(u_sb[:], e_sb[:], ACTF.Exp)
                nc.vector.tensor_mul(
                    u_sb[:],
                    u_sb[:],
                    adj01_sb[:].unsqueeze(1).broadcast_to([N, HEADS, N]),
                )
                s_sb = sbsm.tile([N, HEADS], F32, tag="s")
                nc.vector.tensor_reduce(s_sb[:], u_sb[:], axis=AX.X, op=OP.add)
                r_sb = sbsm.tile([N, HEADS], F32, tag="r")
                nc.vector.reciprocal(r_sb[:], s_sb[:])
                # fold softmax denominator into u before the matmul
                nc.vector.tensor_mul(
                    u_sb[:],
                    u_sb[:],
                    r_sb[:].unsqueeze(2).broadcast_to([N, HEADS, N]),
                )

                # h1T[f,i] per 128-block, via uT (att^T) as moving operand
                uT = sbsm.tile([N, HEADS, N], F16, tag="uT")
                for h in range(HEADS):
                    uT_ps = psT.tile([N, N], F32, tag="tp")
                    nc.tensor.transpose(uT_ps[:], u_sb[:, h, :], ident_sb[:])
                    nc.vector.tensor_copy(uT[:, h, :], uT_ps[:])
                h1T_ps = psH.tile([128, KT2, OUTF], F32, tag="big")
                for h in range(HEADS):
                    for b in range(2):
                        blk = 2 * h + b
                        nc.tensor.matmul(
                            h1T_ps[:, blk, 0:N],
                            g_bf[:, F1 * h + 128 * b:F1 * h + 128 * (b + 1)],
                            uT[:, h, :],
                            start=True,
                            stop=True,
                        )
                # ELU on the transposed h1, emit fp16 for layer-2
                tneg = sbt.tile([128, KT2, N], F32, tag="tneg")
                nc.vector.tensor_scalar_min(tneg[:], h1T_ps[:, :, 0:N], 0.0)
                texp = sbt.tile([128, KT2, N], F32, tag="texp")
                nc.scalar.activation(texp[:], tneg[:], ACTF.Exp)
                tpos = sbt.tile([128, KT2, N], F32, tag="tpos")
                nc.vector.tensor_scalar_max(tpos[:], h1T_ps[:, :, 0:N], 0.0)
                h_sb = sbt.tile([128, KT2, N], F16, tag="h")
                nc.vector.scalar_tensor_tensor(
                    h_sb[:], texp[:], -1.0, tpos[:], op0=OP.add, op1=OP.add
                )

                # ---- phase D: layer-2 GEMM + 1-head attention + MLP ----
                g2_ps = psH.tile([N, OUTF], F32, tag="big")
                for k in range(KT2):
                    nc.tensor.matmul(
                        g2_ps[:],
                        h_sb[:, k, :],
                        w2_sb[:, k, :],
                        start=(k == 0),
                        stop=(k == KT2 - 1),
                    )
                g2_sb = sbsm.tile([N, OUTF], F32, tag="g2")
                nc.vector.tensor_copy(g2_sb[:], g2_ps[:])
                g2b = sbsm.tile([N, OUTF], F16, tag="g2b")
                nc.vector.tensor_copy(g2b[:], g2_ps[:])

                t2 = sbsm.tile([N, OUTF], F32, tag="t2")
                nc.vector.tensor_mul(t2[:], g2_sb[:], a2sb[:])
                e2s = sbsm.tile([N, 1], F32, tag="e2s")
                nc.vector.tensor_reduce(e2s[:], t2[:], axis=AX.X, op=OP.add)
                nc.vector.tensor_mul(t2[:], g2_sb[:], a2db[:])
                e2d = sbsm.tile([N, 1], F32, tag="e2d")
                nc.vector.tensor_reduce(e2d[:], t2[:], axis=AX.X, op=OP.add)

                e2dT_ps = psS.tile([1, N], F32, tag="er")
                nc.tensor.transpose(e2dT_ps[:], e2d[:], ident_sb[:])
                e2dT = sbsm.tile([1, N], F32, tag="e2dT")
                nc.vector.tensor_copy(e2dT[:], e2dT_ps[:])
                ebc2_ps = psS.tile([N, N], F32, tag="er")
                nc.tensor.matmul(
                    ebc2_ps[:], ones46_sb[:], e2dT[:], start=True, stop=True
                )
                e2_sb = sbsm.tile([N, N], F32, tag="e2")
                nc.vector.tensor_scalar(
                    e2_sb[:], ebc2_ps[:], e2s[:, 0:1], None, OP.add
                )
                t22 = sbsm.tile([N, N], F32, tag="t22")
                nc.vector.tensor_scalar_mul(t22[:], e2_sb[:], 0.2)
                nc.vector.tensor_max(e2_sb[:], e2_sb[:], t22[:])
                u2_sb = sbsm.tile([N, N], F32, tag="u2")
                nc.scalar.activation(u2_sb[:], e2_sb[:], ACTF.Exp)
                nc.vector.tensor_mul(u2_sb[:], u2_sb[:], adj01_sb[:])
                s2_sb = sbsm.tile([N, 1], F32, tag="s2")
                nc.vector.tensor_reduce(s2_sb[:], u2_sb[:], axis=AX.X, op=OP.add)
                r2_sb = sbsm.tile([N, 1], F32, tag="r2")
                nc.vector.reciprocal(r2_sb[:], s2_sb[:])
                nc.vector.tensor_scalar(
                    u2_sb[:], u2_sb[:], r2_sb[:, 0:1], None, OP.mult
                )
                u2T_ps = psT.tile([N, N], F32, tag="tp")
                nc.tensor.transpose(u2T_ps[:], u2_sb[:], ident_sb[:])
                u2T = sbsm.tile([N, N], F16, tag="u2T")
                nc.vector.tensor_copy(u2T[:], u2T_ps[:])
                o2_ps = psH.tile([N, OUTF], F32, tag="big")
                nc.tensor.matmul(o2_ps[:], u2T[:], g2b[:], start=True, stop=True)
                # mean over the 64 features folded into host-prescaled mw1 (/64)
                m_sb = sbsm.tile([N, 1], F32, tag="m")
                nc.vector.tensor_reduce(m_sb[:], o2_ps[:], axis=AX.X, op=OP.add)

                z1_ps = psS.tile([1, 12], F32, tag="er")
                nc.tensor.matmul(z1_ps[:], m_sb[:], mw1_sb[:], start=True, stop=True)
                z1_sb = sbsm.tile([1, 12], F32, tag="z1")
                nc.vector.tensor_add(z1_sb[:], z1_ps[:], mb1_sb[:])
                zt_sb = sbsm.tile([1, 12], F32, tag="zt")
                nc.vector.tensor_mul(zt_sb[:], z1_sb[:], mw2t_sb[:])
                z2_sb = sbsm.tile([1, 1], F32, tag="z2")
                nc.vector.tensor_reduce(z2_sb[:], zt_sb[:], axis=AX.X, op=OP.add)
                # sigmoid(z2 + mb2) via exp (avoids a Sigmoid table load)
                zb_sb = sbsm.tile([1, 1], F32, tag="zb")
                nc.vector.tensor_add(zb_sb[:], z2_sb[:], mb2_sb[:])
                zn_sb = sbsm.tile([1, 1], F32, tag="zn")
                nc.vector.tensor_scalar_mul(zn_sb[:], zb_sb[:], -1.0)
                ez_sb = sbsm.tile([1, 1], F32, tag="ez")
                nc.scalar.activation(ez_sb[:], zn_sb[:], ACTF.Exp)
                ez1_sb = sbsm.tile([1, 1], F32, tag="ez1")
                nc.vector.tensor_scalar_add(ez1_sb[:], ez_sb[:], 1.0)
                res_sb = sbsm.tile([1, 1], F32, tag="res")
                nc.vector.reciprocal(res_sb[:], ez1_sb[:])
                nc.sync.dma_start(out.ap(), res_sb[:])

    nc.compile()
    return nc


_NC_CACHE = []


def _get_nc():
    if not _NC_CACHE:
        _NC_CACHE.append(build())
    return _NC_CACHE[0]


def _prep_in_maps(x, adj, W1, a1, W2, a2, mw1, mb1, mw2, mb2):
    adj01 = adj[:, :, 0].astype(np.float32)
    shared = {
        "adj01": adj01,
        "asrcf": np.ascontiguousarray(
            a1[:, :F1].reshape(1, HID).astype(np.float16)
        ),
        "adstf": np.ascontiguousarray(
            a1[:, F1:].reshape(1, HID).astype(np.float16)
        ),
        "a2sf": np.ascontiguousarray(a2[0, :OUTF].reshape(1, OUTF).astype(np.float16)),
        "a2df": np.ascontiguousarray(a2[0, OUTF:].reshape(1, OUTF).astype(np.float16)),
        "w2r": np.ascontiguousarray(
            W2.reshape(KT2, 128, OUTF).transpose(1, 0, 2).reshape(128, KT2 * OUTF)
        ).astype(np.float16),
        "sel": np.kron(np.eye(HEADS, dtype=np.float32), np.ones((1, N), np.float32)),
        "ones46": np.ones((1, N), np.float32),
        "mw1": np.ascontiguousarray(mw1 / np.float32(OUTF)),
        "mb1": mb1.reshape(1, 12).astype(np.float32),
        "mw2t": np.ascontiguousarray(mw2.reshape(1, 12)),
        "mb2": mb2.reshape(1, 1).astype(np.float32),
        "ident": np.eye(N, dtype=np.float32),
    }
    x8 = (x * np.float32(X_SCALE)).astype(NP_F8)
    W18 = (W1 * np.float32(W_SCALE)).astype(NP_F8)
    in_maps = []
    for c in range(NCORES):
        m = dict(shared)
        # x slice, transposed to [128, kt, 46]
        xc = np.ascontiguousarray(x8[:, KC * c:KC * (c + 1)]).T  # [16384, 46]
        m["xs"] = np.ascontiguousarray(
            xc.reshape(KT, 128, N).transpose(1, 0, 2).reshape(128, KT * N)
        )
        # W1 slice, [128, G, KT, GW] flattened
        w1c = W18[KC * c:KC * (c + 1), :].reshape(KT, 128, G, GW)
        m["w1"] = np.ascontiguousarray(
            w1c.transpose(1, 2, 0, 3).reshape(128, G * KT * GW)
        )
        in_maps.append(m)
    return in_maps


def kernel(**inputs):
    x = np.asarray(inputs["x"], dtype=np.float32)
    adj = np.asarray(inputs["adj_mat"]).astype(bool).reshape(N, N, 1)
    W1 = np.asarray(inputs["W1"], dtype=np.float32)
    a1 = np.asarray(inputs["a1"], dtype=np.float32)
    W2 = np.asarray(inputs["W2"], dtype=np.float32)
    a2 = np.asarray(inputs["a2"], dtype=np.float32)
    mw1 = np.asarray(inputs["mlp_w1"], dtype=np.float32)
    mb1 = np.asarray(inputs["mlp_b1"], dtype=np.float32)
    mw2 = np.asarray(inputs["mlp_w2"], dtype=np.float32)
    mb2 = np.asarray(inputs["mlp_b2"], dtype=np.float32)

    nc = _get_nc()
    in_maps = _prep_in_maps(x, adj, W1, a1, W2, a2, mw1, mb1, mw2, mb2)
    res = run_bass_kernel_spmd(nc, in_maps, core_ids=list(range(NCORES)))
    return res.results[0]["out"].reshape(1).astype(np.float32)


# revision 10
# speedup vs baseline: 1.1156x; 1.1156x over previous
"""Bass/Trainium2 kernel for nn_GAT_25082609009415.

GAT: g = x[46,131072] @ W1[131072,2048] -> 8-head masked attention ->
ELU -> h @ W2[2048,64] -> 1-head attention -> mean -> MLP(46->12->1) -> sigmoid.

Strategy (8 NeuronCores), v2:
  * K-shard the dominant GEMM: core c streams W1[16384c:16384(c+1), :]
    in fp8e4 (host-scaled by 2^12; x scaled by 2^5) -- 33.5 MB/core,
    4x fewer HBM bytes than the fp32 baseline.  x arrives
    host-pretransposed as [128, kt, 46] fp8, so no on-device transposes.
  * W1 is laid out in G=4 column groups of 512; each group's partial
    g accumulates in its own PSUM bank and is AllReduced while the next
    group's GEMM streams -- 3 of 4 collectives are hidden.
  * e_src/e_dst are linear in g, so their per-core partials are computed
    from the local partial g and ride along in the last AllReduce.
  * Tail attention avoids gT entirely: e_src/e_dst via vector
    mult+reduce, e_dst broadcast via tiny PE outer-product matmuls,
    h1 computed directly transposed (h1T) so layer-2 needs no
    transposes.  Single activation table (Exp), preloaded during the
    GEMM; sigmoid is computed via exp+reciprocal.
"""
import numpy as np
import ml_dtypes

import concourse.bass as bass
import concourse.bacc as bacc
import concourse.tile as tile
from concourse import mybir
from concourse.bass_utils import run_bass_kernel_spmd

N = 46
KTOT = 131072
HID = 2048
HEADS = 8
F1 = HID // HEADS          # 256 features / head
OUTF = 64
NCORES = 8
KC = KTOT // NCORES        # 16384 contraction elems per core
KT = KC // 128             # 128 k-tiles per core
KT2 = HID // 128           # 16 k-tiles for layer-2 GEMM
G = 4                      # column groups for pipelined AllReduce
GW = HID // G              # 512 columns per group
HPG = HEADS // G           # heads per group
TPD = 32                   # k-tiles per W1 DMA chunk (16KB/partition)
NCH = KT // TPD            # chunks per group

W_SCALE = float(2 ** 12)   # keeps W1 (~+-0.0028) in fp8e4 normal range
X_SCALE = float(2 ** 5)    # keeps x (~N(0,1)) well under fp8e4 max 240
DESCALE = 1.0 / (W_SCALE * X_SCALE)

F32 = mybir.dt.float32
F16 = mybir.dt.float16
F8 = mybir.dt.float8e4
AX = mybir.AxisListType
OP = mybir.AluOpType
ACTF = mybir.ActivationFunctionType

NP_F8 = ml_dtypes.float8_e4m3


def build():
    nc = bacc.Bacc(
        "TRN2",
        target_bir_lowering=False,
        debug=False,
        enable_asserts=False,
        num_devices=NCORES,
    )
    xs = nc.dram_tensor("xs", [128, KT * N], F8, kind="ExternalInput")
    w1 = nc.dram_tensor("w1", [128, G * KT * GW], F8, kind="ExternalInput")
    w2r = nc.dram_tensor("w2r", [128, KT2 * OUTF], F16, kind="ExternalInput")
    adj01 = nc.dram_tensor("adj01", [N, N], F32, kind="ExternalInput")
    asrcf = nc.dram_tensor("asrcf", [1, HID], F16, kind="ExternalInput")
    adstf = nc.dram_tensor("adstf", [1, HID], F16, kind="ExternalInput")
    a2sf = nc.dram_tensor("a2sf", [1, OUTF], F16, kind="ExternalInput")
    a2df = nc.dram_tensor("a2df", [1, OUTF], F16, kind="ExternalInput")
    sel = nc.dram_tensor("sel", [HEADS, HEADS * N], F32, kind="ExternalInput")
    ones46 = nc.dram_tensor("ones46", [1, N], F32, kind="ExternalInput")
    mw1 = nc.dram_tensor("mw1", [N, 12], F32, kind="ExternalInput")
    mb1 = nc.dram_tensor("mb1", [1, 12], F32, kind="ExternalInput")
    mw2t = nc.dram_tensor("mw2t", [1, 12], F32, kind="ExternalInput")
    mb2 = nc.dram_tensor("mb2", [1, 1], F32, kind="ExternalInput")
    ident = nc.dram_tensor("ident", [N, N], F32, kind="ExternalInput")
    out = nc.dram_tensor("out", [1, 1], F32, kind="ExternalOutput")

    # width of the last collective: g columns + esrc[8] + edst[8]
    W3 = GW + 2 * HEADS

    with tile.TileContext(nc) as tc:
        with (
            tc.tile_pool(name="psT", bufs=2, space="PSUM") as psT,
            tc.tile_pool(name="const", bufs=1) as cst,
            tc.tile_pool(name="sbxT", bufs=1) as sbxT,
            tc.tile_pool(name="sbw1", bufs=5) as sbw1,
            tc.tile_pool(name="sbst", bufs=2) as sbst,
            tc.tile_pool(name="sbg", bufs=1) as sbg,
            tc.tile_pool(name="sbt", bufs=1) as sbt,
            tc.tile_pool(name="sbsm", bufs=1) as sbsm,
            tc.tile_pool(name="dram", bufs=1, space="DRAM") as dram,
        ):
            # ---- phase A: x slices (pretransposed on host) + first W1 chunk
            # first, so the GEMM can start ASAP; constants go to the vector
            # queue to keep sync/scalar clear for the W1 stream.
            xT = sbxT.tile([128, KT, N], F8, tag="xT")
            XCH = KT // 4
            nc.sync.dma_start(xT[:, 0:XCH, :], xs.ap()[:, 0:XCH * N])
            w1c0 = sbw1.tile([128, TPD, GW], F8, tag="w1")
            nc.scalar.dma_start(w1c0[:], w1.ap()[:, 0:TPD * GW])
            for c in range(1, 4):
                q = nc.sync if c % 2 == 0 else nc.scalar
                q.dma_start(
                    xT[:, XCH * c:XCH * (c + 1), :],
                    xs.ap()[:, XCH * N * c:XCH * N * (c + 1)],
                )

            ident_sb = cst.tile([N, N], F32, tag="ident")
            nc.gpsimd.dma_start(ident_sb[:], ident.ap())
            adj01_sb = cst.tile([N, N], F32, tag="adj01")
            nc.gpsimd.dma_start(adj01_sb[:], adj01.ap())
            sel_sb = cst.tile([HEADS, HEADS * N], F32, tag="sel")
            nc.gpsimd.dma_start(sel_sb[:], sel.ap())
            ones46_sb = cst.tile([1, N], F32, tag="ones46")
            nc.gpsimd.dma_start(ones46_sb[:], ones46.ap())
            w2_sb = cst.tile([128, KT2, OUTF], F16, tag="w2")
            nc.gpsimd.dma_start(w2_sb[:], w2r.ap())
            asrc1 = cst.tile([1, HID], F16, tag="asrc1")
            nc.gpsimd.dma_start(asrc1[:], asrcf.ap())
            adst1 = cst.tile([1, HID], F16, tag="adst1")
            nc.gpsimd.dma_start(adst1[:], adstf.ap())
            a2s1 = cst.tile([1, OUTF], F16, tag="a2s1")
            nc.gpsimd.dma_start(a2s1[:], a2sf.ap())
            a2d1 = cst.tile([1, OUTF], F16, tag="a2d1")
            nc.gpsimd.dma_start(a2d1[:], a2df.ap())
            mw1_sb = cst.tile([N, 12], F32, tag="mw1")
            nc.gpsimd.dma_start(mw1_sb[:], mw1.ap())
            mb1_sb = cst.tile([1, 12], F32, tag="mb1")
            nc.gpsimd.dma_start(mb1_sb[:], mb1.ap())
            mw2t_sb = cst.tile([1, 12], F32, tag="mw2t")
            nc.gpsimd.dma_start(mw2t_sb[:], mw2t.ap())
            mb2_sb = cst.tile([1, 1], F32, tag="mb2")
            nc.gpsimd.dma_start(mb2_sb[:], mb2.ap())

            # broadcast attention vectors across the 46 node partitions
            asrcb = cst.tile([N, HID], F16, tag="asrcb")
            nc.gpsimd.partition_broadcast(asrcb[:], asrc1[:])
            adstb = cst.tile([N, HID], F16, tag="adstb")
            nc.gpsimd.partition_broadcast(adstb[:], adst1[:])
            a2sb = cst.tile([N, OUTF], F16, tag="a2sb")
            nc.gpsimd.partition_broadcast(a2sb[:], a2s1[:])
            a2db = cst.tile([N, OUTF], F16, tag="a2db")
            nc.gpsimd.partition_broadcast(a2db[:], a2d1[:])

            # preload the Exp activation table off the critical path
            dummy = sbsm.tile([1, 1], F32, tag="dummy")
            nc.scalar.activation(dummy[:], mb2_sb[:], ACTF.Exp)

            # tiny dummy collective: absorbs the cross-core startup skew and
            # CC-core wakeup latency while the GEMM streams
            dcc_in = dram.tile([1, 16], F32, tag="dccin")
            dcc_out = dram.tile([1, 16], F32, tag="dccout")
            nc.sync.dma_start(dcc_in[:], ones46_sb[:, 0:16])
            nc.gpsimd.collective_compute(
                "AllReduce",
                OP.add,
                replica_groups=[list(range(NCORES))],
                ins=[dcc_in[:].opt()],
                outs=[dcc_out[:].opt()],
            )

            # staged partial g + esrc/edst partials; AllReduce split 3+1:
            # groups 0-2 fire once the W1 stream is done (hidden under
            # group-3 matmuls), group 3 + e-partials at the end.
            WB = GW + 2 * HEADS
            stageA = sbt.tile([N, 3 * GW], F32, tag="stageA")
            stageB = sbt.tile([N, WB], F32, tag="stageB")
            cc_inA = dram.tile([N, 3 * GW], F32, tag="ccinA")
            cc_outA = dram.tile([N, 3 * GW], F32, tag="ccoutA")
            cc_inB = dram.tile([N, WB], F32, tag="ccinB")
            cc_outB = dram.tile([N, WB], F32, tag="ccoutB")
            g_bf = sbg.tile([N, HID], F16, tag="gbf")
            gfA = sbt.tile([N, 3 * GW], F32, tag="gfA")
            gfB = sbt.tile([N, WB], F32, tag="gfB")

            # ---- phase B: fp8 GEMM in G column groups ----
            with tc.tile_pool(name="psG", bufs=2, space="PSUM") as psG:
                ch = 0
                for g in range(G):
                    gps = psG.tile([N, GW], F32, tag="gps")
                    for c2 in range(NCH):
                        if g == 0 and c2 == 0:
                            w1c = w1c0
                        else:
                            w1c = sbw1.tile([128, TPD, GW], F8, tag="w1")
                            off = (g * KT + c2 * TPD) * GW
                            q = nc.sync if ch % 2 == 1 else nc.scalar
                            q.dma_start(w1c[:], w1.ap()[:, off:off + TPD * GW])
                        ch += 1
                        for t in range(TPD):
                            k = c2 * TPD + t
                            nc.tensor.matmul(
                                gps[:],
                                xT[:, k, :],
                                w1c[:, t, :],
                                start=(k == 0),
                                stop=(k == KT - 1),
                            )
                    # descale partial g out of PSUM
                    if g < G - 1:
                        st_ap = stageA[:, GW * g:GW * (g + 1)]
                    else:
                        st_ap = stageB[:, 0:GW]
                    nc.vector.tensor_scalar_mul(st_ap, gps[:], DESCALE)

                    # partial e_src/e_dst for this group's 2 heads
                    tmp = sbst.tile([N, GW], F32, tag="etmp")
                    nc.vector.tensor_mul(
                        tmp[:], st_ap, asrcb[:, GW * g:GW * (g + 1)]
                    )
                    nc.vector.tensor_reduce(
                        stageB[:, GW + HPG * g:GW + HPG * (g + 1)],
                        tmp[:].rearrange("p (h f) -> p h f", h=HPG),
                        axis=AX.X,
                        op=OP.add,
                    )
                    nc.vector.tensor_mul(
                        tmp[:], st_ap, adstb[:, GW * g:GW * (g + 1)]
                    )
                    nc.vector.tensor_reduce(
                        stageB[:, GW + HEADS + HPG * g:GW + HEADS + HPG * (g + 1)],
                        tmp[:].rearrange("p (h f) -> p h f", h=HPG),
                        axis=AX.X,
                        op=OP.add,
                    )
                    if g == G - 2:
                        # groups 0-2 done: W1 stream has drained, AllReduce
                        # them now, hidden under group-3 matmuls
                        nc.sync.dma_start(cc_inA[:], stageA[:])
                        nc.gpsimd.collective_compute(
                            "AllReduce",
                            OP.add,
                            replica_groups=[list(range(NCORES))],
                            ins=[cc_inA[:].opt()],
                            outs=[cc_outA[:].opt()],
                        )
                        nc.sync.dma_start(gfA[:], cc_outA[:])
                        nc.vector.tensor_copy(g_bf[:, 0:3 * GW], gfA[:])

            nc.sync.dma_start(cc_inB[:], stageB[:])
            nc.gpsimd.collective_compute(
                "AllReduce",
                OP.add,
                replica_groups=[list(range(NCORES))],
                ins=[cc_inB[:].opt()],
                outs=[cc_outB[:].opt()],
            )
            nc.sync.dma_start(gfB[:], cc_outB[:])
            nc.vector.tensor_copy(g_bf[:, 3 * GW:HID], gfB[:, 0:GW])

            esrc = gfB[:, GW:GW + HEADS]               # [46, 8] fp32
            edst = gfB[:, GW + HEADS:WB]               # [46, 8] fp32

            with (
                tc.tile_pool(name="psE", bufs=1, space="PSUM") as psE,
                tc.tile_pool(name="psH", bufs=1, space="PSUM") as psH,
                tc.tile_pool(name="psS", bufs=1, space="PSUM") as psS,
            ):
                # ---- phase C: layer-1 attention (8 heads) ----
                # edst[j,h] -> edstT[h,j] -> broadcast over i via sel matmuls
                edT_ps = psT.tile([HEADS, N], F32, tag="tp")
                nc.tensor.transpose(edT_ps[:], edst, ident_sb[:])
                edT = sbsm.tile([HEADS, N], F32, tag="edT")
                nc.vector.tensor_copy(edT[:], edT_ps[:])
                ebc_ps = psE.tile([N, HEADS, N], F32, tag="ebc")
                for h in range(HEADS):
                    nc.tensor.matmul(
                        ebc_ps[:, h, :],
                        sel_sb[:, N * h:N * (h + 1)],
                        edT[:],
                        start=True,
                        stop=True,
                    )
                # e = leaky_relu(esrc_i + edst_j, 0.2); u = exp(e) * adj
                e_sb = sbsm.tile([N, HEADS, N], F32, tag="e")
                nc.vector.tensor_add(
                    e_sb[:],
                    ebc_ps[:],
                    esrc.unsqueeze(2).broadcast_to([N, HEADS, N]),
                )
                t02 = sbsm.tile([N, HEADS, N], F32, tag="t02")
                nc.vector.tensor_scalar_mul(t02[:], e_sb[:], 0.2)
                nc.vector.tensor_max(e_sb[:], e_sb[:], t02[:])
                u_sb = sbsm.tile([N, HEADS, N], F32, tag="u")
                nc.scalar.activation(u_sb[:], e_sb[:], ACTF.Exp)
                nc.vector.tensor_mul(
                    u_sb[:],
                    u_sb[:],
                    adj01_sb[:].unsqueeze(1).broadcast_to([N, HEADS, N]),
                )
                s_sb = sbsm.tile([N, HEADS], F32, tag="s")
                nc.vector.tensor_reduce(s_sb[:], u_sb[:], axis=AX.X, op=OP.add)
                r_sb = sbsm.tile([N, HEADS], F32, tag="r")
                nc.vector.reciprocal(r_sb[:], s_sb[:])
                # fold softmax denominator into u before the matmul
                nc.vector.tensor_mul(
                    u_sb[:],
                    u_sb[:],
                    r_sb[:].unsqueeze(2).broadcast_to([N, HEADS, N]),
                )

                # h1T[f,i] per 128-block, via uT (att^T) as moving operand
                uT = sbsm.tile([N, HEADS, N], F16, tag="uT")
                for h in range(HEADS):
                    uT_ps = psT.tile([N, N], F32, tag="tp")
                    nc.tensor.transpose(uT_ps[:], u_sb[:, h, :], ident_sb[:])
                    nc.vector.tensor_copy(uT[:, h, :], uT_ps[:])
                h1T_ps = psH.tile([128, KT2, OUTF], F32, tag="big")
                for h in range(HEADS):
                    for b in range(2):
                        blk = 2 * h + b
                        nc.tensor.matmul(
                            h1T_ps[:, blk, 0:N],
                            g_bf[:, F1 * h + 128 * b:F1 * h + 128 * (b + 1)],
                            uT[:, h, :],
                            start=True,
                            stop=True,
                        )
                # ELU on the transposed h1, emit fp16 for layer-2
                tneg = sbt.tile([128, KT2, N], F32, tag="tneg")
                nc.vector.tensor_scalar_min(tneg[:], h1T_ps[:, :, 0:N], 0.0)
                texp = sbt.tile([128, KT2, N], F32, tag="texp")
                nc.scalar.activation(texp[:], tneg[:], ACTF.Exp)
                tpos = sbt.tile([128, KT2, N], F32, tag="tpos")
                nc.vector.tensor_scalar_max(tpos[:], h1T_ps[:, :, 0:N], 0.0)
                h_sb = sbt.tile([128, KT2, N], F16, tag="h")
                nc.vector.scalar_tensor_tensor(
                    h_sb[:], texp[:], -1.0, tpos[:], op0=OP.add, op1=OP.add
                )

                # ---- phase D: layer-2 GEMM + 1-head attention + MLP ----
                g2_ps = psH.tile([N, OUTF], F32, tag="big")
                for k in range(KT2):
                    nc.tensor.matmul(
                        g2_ps[:],
                        h_sb[:, k, :],
                        w2_sb[:, k, :],
                        start=(k == 0),
                        stop=(k == KT2 - 1),
                    )
                g2_sb = sbsm.tile([N, OUTF], F32, tag="g2")
                nc.vector.tensor_copy(g2_sb[:], g2_ps[:])
                g2b = sbsm.tile([N, OUTF], F16, tag="g2b")
                nc.vector.tensor_copy(g2b[:], g2_ps[:])

                t2 = sbsm.tile([N, OUTF], F32, tag="t2")
                nc.vector.tensor_mul(t2[:], g2_sb[:], a2sb[:])
                e2s = sbsm.tile([N, 1], F32, tag="e2s")
                nc.vector.tensor_reduce(e2s[:], t2[:], axis=AX.X, op=OP.add)
                nc.vector.tensor_mul(t2[:], g2_sb[:], a2db[:])
                e2d = sbsm.tile([N, 1], F32, tag="e2d")
                nc.vector.tensor_reduce(e2d[:], t2[:], axis=AX.X, op=OP.add)

                e2dT_ps = psS.tile([1, N], F32, tag="er")
                nc.tensor.transpose(e2dT_ps[:], e2d[:], ident_sb[:])
                e2dT = sbsm.tile([1, N], F32, tag="e2dT")
                nc.vector.tensor_copy(e2dT[:], e2dT_ps[:])
                ebc2_ps = psS.tile([N, N], F32, tag="er")
                nc.tensor.matmul(
                    ebc2_ps[:], ones46_sb[:], e2dT[:], start=True, stop=True
                )
                e2_sb = sbsm.tile([N, N], F32, tag="e2")
                nc.vector.tensor_scalar(
                    e2_sb[:], ebc2_ps[:], e2s[:, 0:1], None, OP.add
                )
                t22 = sbsm.tile([N, N], F32, tag="t22")
                nc.vector.tensor_scalar_mul(t22[:], e2_sb[:], 0.2)
                nc.vector.tensor_max(e2_sb[:], e2_sb[:], t22[:])
                u2_sb = sbsm.tile([N, N], F32, tag="u2")
                nc.scalar.activation(u2_sb[:], e2_sb[:], ACTF.Exp)
                nc.vector.tensor_mul(u2_sb[:], u2_sb[:], adj01_sb[:])
                s2_sb = sbsm.tile([N, 1], F32, tag="s2")
                nc.vector.tensor_reduce(s2_sb[:], u2_sb[:], axis=AX.X, op=OP.add)
                r2_sb = sbsm.tile([N, 1], F32, tag="r2")
                nc.vector.reciprocal(r2_sb[:], s2_sb[:])
                nc.vector.tensor_scalar(
                    u2_sb[:], u2_sb[:], r2_sb[:, 0:1], None, OP.mult
                )
                u2T_ps = psT.tile([N, N], F32, tag="tp")
                nc.tensor.transpose(u2T_ps[:], u2_sb[:], ident_sb[:])
                u2T = sbsm.tile([N, N], F16, tag="u2T")
                nc.vector.tensor_copy(u2T[:], u2T_ps[:])
                o2_ps = psH.tile([N, OUTF], F32, tag="big")
                nc.tensor.matmul(o2_ps[:], u2T[:], g2b[:], start=True, stop=True)
                # mean over the 64 features folded into host-prescaled mw1 (/64)
                m_sb = sbsm.tile([N, 1], F32, tag="m")
                nc.vector.tensor_reduce(m_sb[:], o2_ps[:], axis=AX.X, op=OP.add)

                z1_ps = psS.tile([1, 12], F32, tag="er")
                nc.tensor.matmul(z1_ps[:], m_sb[:], mw1_sb[:], start=True, stop=True)
                z1_sb = sbsm.tile([1, 12], F32, tag="z1")
                nc.vector.tensor_add(z1_sb[:], z1_ps[:], mb1_sb[:])
                zt_sb = sbsm.tile([1, 12], F32, tag="zt")
                nc.vector.tensor_mul(zt_sb[:], z1_sb[:], mw2t_sb[:])
                z2_sb = sbsm.tile([1, 1], F32, tag="z2")
                nc.vector.tensor_reduce(z2_sb[:], zt_sb[:], axis=AX.X, op=OP.add)
                # sigmoid(z2 + mb2) via exp (avoids a Sigmoid table load)
                zb_sb = sbsm.tile([1, 1], F32, tag="zb")
                nc.vector.tensor_add(zb_sb[:], z2_sb[:], mb2_sb[:])
                zn_sb = sbsm.tile([1, 1], F32, tag="zn")
                nc.vector.tensor_scalar_mul(zn_sb[:], zb_sb[:], -1.0)
                ez_sb = sbsm.tile([1, 1], F32, tag="ez")
                nc.scalar.activation(ez_sb[:], zn_sb[:], ACTF.Exp)
                ez1_sb = sbsm.tile([1, 1], F32, tag="ez1")
                nc.vector.tensor_scalar_add(ez1_sb[:], ez_sb[:], 1.0)
                res_sb = sbsm.tile([1, 1], F32, tag="res")
                nc.vector.reciprocal(res_sb[:], ez1_sb[:])
                nc.sync.dma_start(out.ap(), res_sb[:])

    nc.compile()
    return nc


_NC_CACHE = []


def _get_nc():
    if not _NC_CACHE:
        _NC_CACHE.append(build())
    return _NC_CACHE[0]


def _prep_in_maps(x, adj, W1, a1, W2, a2, mw1, mb1, mw2, mb2):
    adj01 = adj[:, :, 0].astype(np.float32)
    shared = {
        "adj01": adj01,
        "asrcf": np.ascontiguousarray(
            a1[:, :F1].reshape(1, HID).astype(np.float16)
        ),
        "adstf": np.ascontiguousarray(
            a1[:, F1:].reshape(1, HID).astype(np.float16)
        ),
        "a2sf": np.ascontiguousarray(a2[0, :OUTF].reshape(1, OUTF).astype(np.float16)),
        "a2df": np.ascontiguousarray(a2[0, OUTF:].reshape(1, OUTF).astype(np.float16)),
        "w2r": np.ascontiguousarray(
            W2.reshape(KT2, 128, OUTF).transpose(1, 0, 2).reshape(128, KT2 * OUTF)
        ).astype(np.float16),
        "sel": np.kron(np.eye(HEADS, dtype=np.float32), np.ones((1, N), np.float32)),
        "ones46": np.ones((1, N), np.float32),
        "mw1": np.ascontiguousarray(mw1 / np.float32(OUTF)),
        "mb1": mb1.reshape(1, 12).astype(np.float32),
        "mw2t": np.ascontiguousarray(mw2.reshape(1, 12)),
        "mb2": mb2.reshape(1, 1).astype(np.float32),
        "ident": np.eye(N, dtype=np.float32),
    }
    x8 = (x * np.float32(X_SCALE)).astype(NP_F8)
    W18 = (W1 * np.float32(W_SCALE)).astype(NP_F8)
    in_maps = []
    for c in range(NCORES):
        m = dict(shared)
        # x slice, transposed to [128, kt, 46]
        xc = np.ascontiguousarray(x8[:, KC * c:KC * (c + 1)]).T  # [16384, 46]
        m["xs"] = np.ascontiguousarray(
            xc.reshape(KT, 128, N).transpose(1, 0, 2).reshape(128, KT * N)
        )
        # W1 slice, [128, G, KT, GW] flattened
        w1c = W18[KC * c:KC * (c + 1), :].reshape(KT, 128, G, GW)
        m["w1"] = np.ascontiguousarray(
            w1c.transpose(1, 2, 0, 3).reshape(128, G * KT * GW)
        )
        in_maps.append(m)
    return in_maps


def kernel(**inputs):
    x = np.asarray(inputs["x"], dtype=np.float32)
    adj = np.asarray(inputs["adj_mat"]).astype(bool).reshape(N, N, 1)
    W1 = np.asarray(inputs["W1"], dtype=np.float32)
    a1 = np.asarray(inputs["a1"], dtype=np.float32)
    W2 = np.asarray(inputs["W2"], dtype=np.float32)
    a2 = np.asarray(inputs["a2"], dtype=np.float32)
    mw1 = np.asarray(inputs["mlp_w1"], dtype=np.float32)
    mb1 = np.asarray(inputs["mlp_b1"], dtype=np.float32)
    mw2 = np.asarray(inputs["mlp_w2"], dtype=np.float32)
    mb2 = np.asarray(inputs["mlp_b2"], dtype=np.float32)

    nc = _get_nc()
    in_maps = _prep_in_maps(x, adj, W1, a1, W2, a2, mw1, mb1, mw2, mb2)
    res = run_bass_kernel_spmd(nc, in_maps, core_ids=list(range(NCORES)))
    return res.results[0]["out"].reshape(1).astype(np.float32)


# revision 16
# speedup vs baseline: 1.2602x; 1.1296x over previous
"""Bass/Trainium2 kernel for nn_GAT_25082609009415.

GAT: g = x[46,131072] @ W1[131072,2048] -> 8-head masked attention ->
ELU -> h @ W2[2048,64] -> 1-head attention -> mean -> MLP(46->12->1) -> sigmoid.

Strategy (8 NeuronCores), v4:
  * K-shard the dominant GEMM: core c streams W1[16384c:16384(c+1), :]
    in fp8e4 (host-scaled by 2^12; x scaled by 2^5) -- 33.5 MB/core,
    4x fewer HBM bytes than fp32.  x arrives host-pretransposed as
    [128, kt, 46] fp8, so no on-device transposes.
  * e_src/e_dst are linear in g, so their per-core partials are computed
    from the local partial g and ride along in the AllReduces.
  * The g AllReduce is fp16 and split 3+1 by column group: groups 0-2
    (+ heads 0-5 e-partials) fire once the W1 stream has drained and
    hide under group-3 matmuls; group 3 (+ heads 6-7) fires at the end.
    A tiny dummy AllReduce at kernel start absorbs the cross-core
    startup skew and CC wakeup latency.
  * The tail is emitted per head-range: heads 0-5 attention, their h1T
    blocks, ELU, and 12/16 layer-2 matmuls all run during the second
    AllReduce; only heads 6-7 plus the small layer-2 attention + MLP
    are exposed.  Single activation table (Exp) preloaded during the
    GEMM; sigmoid is computed via exp+reciprocal.
"""
import numpy as np
import ml_dtypes

import concourse.bass as bass
import concourse.bacc as bacc
import concourse.tile as tile
from concourse import mybir
from concourse.bass_utils import run_bass_kernel_spmd

N = 46
KTOT = 131072
HID = 2048
HEADS = 8
F1 = HID // HEADS          # 256 features / head
OUTF = 64
NCORES = 8
KC = KTOT // NCORES        # 16384 contraction elems per core
KT = KC // 128             # 128 k-tiles per core
KT2 = HID // 128           # 16 k-tiles for layer-2 GEMM
G = 4                      # column groups
GW = HID // G              # 512 columns per group
HPG = HEADS // G           # heads per group
TPD = 32                   # k-tiles per W1 DMA chunk (16KB/partition)
NCH = KT // TPD            # chunks per group
HA = 6                     # heads resolved by the first AllReduce

W_SCALE = float(2 ** 12)   # keeps W1 (~+-0.0028) in fp8e4 normal range
X_SCALE = float(2 ** 5)    # keeps x (~N(0,1)) well under fp8e4 max 240
DESCALE = 1.0 / (W_SCALE * X_SCALE)

F32 = mybir.dt.float32
F16 = mybir.dt.float16
F8 = mybir.dt.float8e4
AX = mybir.AxisListType
OP = mybir.AluOpType
ACTF = mybir.ActivationFunctionType

NP_F8 = ml_dtypes.float8_e4m3

WA = 3 * GW + 2 * HA       # first collective: 1536 g cols + 6+6 e-partials
WB = GW + 2 * (HEADS - HA)  # second: 512 g cols + 2+2 e-partials


def build():
    nc = bacc.Bacc(
        "TRN2",
        target_bir_lowering=False,
        debug=False,
        enable_asserts=False,
        num_devices=NCORES,
    )
    xs = nc.dram_tensor("xs", [128, KT * N], F8, kind="ExternalInput")
    w1 = nc.dram_tensor("w1", [128, G * KT * GW], F8, kind="ExternalInput")
    w2r = nc.dram_tensor("w2r", [128, KT2 * OUTF], F16, kind="ExternalInput")
    adj01 = nc.dram_tensor("adj01", [N, N], F32, kind="ExternalInput")
    asrcf = nc.dram_tensor("asrcf", [1, HID], F16, kind="ExternalInput")
    adstf = nc.dram_tensor("adstf", [1, HID], F16, kind="ExternalInput")
    a2sf = nc.dram_tensor("a2sf", [1, OUTF], F16, kind="ExternalInput")
    a2df = nc.dram_tensor("a2df", [1, OUTF], F16, kind="ExternalInput")
    sel = nc.dram_tensor("sel", [HEADS, HEADS * N], F32, kind="ExternalInput")
    ones46 = nc.dram_tensor("ones46", [1, N], F32, kind="ExternalInput")
    mw1 = nc.dram_tensor("mw1", [N, 12], F32, kind="ExternalInput")
    mb1 = nc.dram_tensor("mb1", [1, 12], F32, kind="ExternalInput")
    mw2t = nc.dram_tensor("mw2t", [1, 12], F32, kind="ExternalInput")
    mb2 = nc.dram_tensor("mb2", [1, 1], F32, kind="ExternalInput")
    ident = nc.dram_tensor("ident", [N, N], F32, kind="ExternalInput")
    out = nc.dram_tensor("out", [1, 1], F32, kind="ExternalOutput")

    with tile.TileContext(nc) as tc:
        with (
            tc.tile_pool(name="psT", bufs=2, space="PSUM") as psT,
            tc.tile_pool(name="psE", bufs=1, space="PSUM") as psE,
            tc.tile_pool(name="psH", bufs=1, space="PSUM") as psH,
            tc.tile_pool(name="psO", bufs=1, space="PSUM") as psO,
            tc.tile_pool(name="psS", bufs=1, space="PSUM") as psS,
            tc.tile_pool(name="const", bufs=1) as cst,
            tc.tile_pool(name="sbxT", bufs=1) as sbxT,
            tc.tile_pool(name="sbw1", bufs=4) as sbw1,
            tc.tile_pool(name="sbst", bufs=2) as sbst,
            tc.tile_pool(name="sbt", bufs=1) as sbt,
            tc.tile_pool(name="sbsm", bufs=1) as sbsm,
            tc.tile_pool(name="dram", bufs=1, space="DRAM") as dram,
        ):
            # ---- phase A: x (pretransposed on host) + first W1 chunk first,
            # so the GEMM starts ASAP; constants ride the gpsimd queue.
            xT = sbxT.tile([128, KT, N], F8, tag="xT")
            XCH = KT // 8
            nc.sync.dma_start(xT[:, 0:XCH, :], xs.ap()[:, 0:XCH * N])
            w1c0a = sbw1.tile([128, 8, GW], F8, tag="w1a")
            nc.scalar.dma_start(w1c0a[:], w1.ap()[:, 0:8 * GW])
            w1c0b = sbw1.tile([128, 8, GW], F8, tag="w1b")
            nc.scalar.dma_start(w1c0b[:], w1.ap()[:, 8 * GW:16 * GW])
            w1c0c = sbw1.tile([128, 16, GW], F8, tag="w1c")
            nc.scalar.dma_start(w1c0c[:], w1.ap()[:, 16 * GW:32 * GW])
            for c in range(1, 8):
                q = nc.sync if c % 2 == 0 else nc.scalar
                q.dma_start(
                    xT[:, XCH * c:XCH * (c + 1), :],
                    xs.ap()[:, XCH * N * c:XCH * N * (c + 1)],
                )

            ident_sb = cst.tile([N, N], F32, tag="ident")
            nc.gpsimd.dma_start(ident_sb[:], ident.ap())
            adj01_sb = cst.tile([N, N], F32, tag="adj01")
            nc.gpsimd.dma_start(adj01_sb[:], adj01.ap())
            sel_sb = cst.tile([HEADS, HEADS * N], F32, tag="sel")
            nc.gpsimd.dma_start(sel_sb[:], sel.ap())
            ones46_sb = cst.tile([1, N], F32, tag="ones46")
            nc.gpsimd.dma_start(ones46_sb[:], ones46.ap())
            w2_sb = cst.tile([128, KT2, OUTF], F16, tag="w2")
            nc.gpsimd.dma_start(w2_sb[:], w2r.ap())
            asrc1 = cst.tile([1, HID], F16, tag="asrc1")
            nc.gpsimd.dma_start(asrc1[:], asrcf.ap())
            adst1 = cst.tile([1, HID], F16, tag="adst1")
            nc.gpsimd.dma_start(adst1[:], adstf.ap())
            a2s1 = cst.tile([1, OUTF], F16, tag="a2s1")
            nc.gpsimd.dma_start(a2s1[:], a2sf.ap())
            a2d1 = cst.tile([1, OUTF], F16, tag="a2d1")
            nc.gpsimd.dma_start(a2d1[:], a2df.ap())
            mw1_sb = cst.tile([N, 12], F32, tag="mw1")
            nc.gpsimd.dma_start(mw1_sb[:], mw1.ap())
            mb1_sb = cst.tile([1, 12], F32, tag="mb1")
            nc.gpsimd.dma_start(mb1_sb[:], mb1.ap())
            mw2t_sb = cst.tile([1, 12], F32, tag="mw2t")
            nc.gpsimd.dma_start(mw2t_sb[:], mw2t.ap())
            mb2_sb = cst.tile([1, 1], F32, tag="mb2")
            nc.gpsimd.dma_start(mb2_sb[:], mb2.ap())

            # broadcast attention vectors across the 46 node partitions
            asrcb = cst.tile([N, HID], F16, tag="asrcb")
            nc.gpsimd.partition_broadcast(asrcb[:], asrc1[:])
            adstb = cst.tile([N, HID], F16, tag="adstb")
            nc.gpsimd.partition_broadcast(adstb[:], adst1[:])
            a2sb = cst.tile([N, OUTF], F16, tag="a2sb")
            nc.gpsimd.partition_broadcast(a2sb[:], a2s1[:])
            a2db = cst.tile([N, OUTF], F16, tag="a2db")
            nc.gpsimd.partition_broadcast(a2db[:], a2d1[:])

            # preload the Exp activation table off the critical path
            dummy = sbsm.tile([1, 1], F32, tag="dummy")
            nc.scalar.activation(dummy[:], mb2_sb[:], ACTF.Exp)

            # tiny dummy collective: absorbs cross-core startup skew and
            # CC wakeup latency while the GEMM streams
            dcc_in = dram.tile([1, 16], F32, tag="dccin")
            dcc_out = dram.tile([1, 16], F32, tag="dccout")
            nc.gpsimd.dma_start(dcc_in[:], ones46_sb[:, 0:16])
            nc.gpsimd.collective_compute(
                "AllReduce",
                OP.add,
                replica_groups=[list(range(NCORES))],
                ins=[dcc_in[:].opt()],
                outs=[dcc_out[:].opt()],
            )

            stageA = sbt.tile([N, WA], F16, tag="stageA")
            stageB = sbt.tile([N, WB], F16, tag="stageB")
            cc_inA = dram.tile([N, WA], F16, tag="ccinA")
            cc_outA = dram.tile([N, WA], F16, tag="ccoutA")
            cc_inB = dram.tile([N, WB], F16, tag="ccinB")
            cc_outB = dram.tile([N, WB], F16, tag="ccoutB")
            gfA = sbt.tile([N, WA], F16, tag="gfA")
            gfB = sbt.tile([N, WB], F16, tag="gfB")

            # ---- phase B: fp8 GEMM in G column groups ----
            with tc.tile_pool(name="psG", bufs=2, space="PSUM") as psG:
                ch = 0
                for g in range(G):
                    gps = psG.tile([N, GW], F32, tag="gps")
                    sizes = [8, 8, 16] + [TPD] * 3 if g == 0 else [TPD] * NCH
                    kt0 = 0
                    for ci, sz in enumerate(sizes):
                        if g == 0 and ci == 0:
                            w1c = w1c0a
                        elif g == 0 and ci == 1:
                            w1c = w1c0b
                        elif g == 0 and ci == 2:
                            w1c = w1c0c
                        else:
                            w1c = sbw1.tile([128, sz, GW], F8, tag="w1")
                            off = (g * KT + kt0) * GW
                            q = nc.sync if ch % 2 == 1 else nc.scalar
                            q.dma_start(w1c[:], w1.ap()[:, off:off + sz * GW])
                        ch += 1
                        for t in range(sz):
                            k = kt0 + t
                            nc.tensor.matmul(
                                gps[:],
                                xT[:, k, :],
                                w1c[:, t, :],
                                start=(k == 0),
                                stop=(k == KT - 1),
                            )
                        kt0 += sz
                    # descale partial g out of PSUM (fp16 for the collective)
                    if g < G - 1:
                        st_ap = stageA[:, GW * g:GW * (g + 1)]
                        es_ap = stageA[:, 3 * GW + HPG * g:3 * GW + HPG * (g + 1)]
                        ed_ap = stageA[
                            :, 3 * GW + HA + HPG * g:3 * GW + HA + HPG * (g + 1)
                        ]
                    else:
                        st_ap = stageB[:, 0:GW]
                        es_ap = stageB[:, GW:GW + HPG]
                        ed_ap = stageB[:, GW + HPG:WB]
                    nc.vector.tensor_scalar_mul(st_ap, gps[:], DESCALE)

                    # partial e_src/e_dst for this group's 2 heads
                    tmp = sbst.tile([N, GW], F32, tag="etmp")
                    nc.vector.tensor_mul(
                        tmp[:], st_ap, asrcb[:, GW * g:GW * (g + 1)]
                    )
                    with nc.allow_low_precision(
                        reason="fp16 e-partials; final scalar tolerates it"
                    ):
                        nc.vector.tensor_reduce(
                            es_ap,
                            tmp[:].rearrange("p (h f) -> p h f", h=HPG),
                            axis=AX.X,
                            op=OP.add,
                        )
                        nc.vector.tensor_mul(
                            tmp[:], st_ap, adstb[:, GW * g:GW * (g + 1)]
                        )
                        nc.vector.tensor_reduce(
                            ed_ap,
                            tmp[:].rearrange("p (h f) -> p h f", h=HPG),
                            axis=AX.X,
                            op=OP.add,
                        )
                    if g == G - 2:
                        # groups 0-2 done and W1 stream drained: AllReduce
                        # them now, hidden under group-3 matmuls
                        nc.gpsimd.dma_start(cc_inA[:], stageA[:])
                        nc.gpsimd.collective_compute(
                            "AllReduce",
                            OP.add,
                            replica_groups=[list(range(NCORES))],
                            ins=[cc_inA[:].opt()],
                            outs=[cc_outA[:].opt()],
                        )
                        nc.gpsimd.dma_start(gfA[:], cc_outA[:])

            nc.gpsimd.dma_start(cc_inB[:], stageB[:])
            nc.gpsimd.collective_compute(
                "AllReduce",
                OP.add,
                replica_groups=[list(range(NCORES))],
                ins=[cc_inB[:].opt()],
                outs=[cc_outB[:].opt()],
            )
            nc.gpsimd.dma_start(gfB[:], cc_outB[:])

            # ---- tail: per head-range; heads 0-5 overlap the 2nd AllReduce
            ebc_ps = psE.tile([N, HEADS, N], F32, tag="ebc")
            e_sb = sbsm.tile([N, HEADS, N], F32, tag="e")
            t02 = sbsm.tile([N, HEADS, N], F32, tag="t02")
            u_sb = sbsm.tile([N, HEADS, N], F32, tag="u")
            s_sb = sbsm.tile([N, HEADS], F32, tag="s")
            r_sb = sbsm.tile([N, HEADS], F32, tag="r")
            uT = sbsm.tile([N, HEADS, N], F16, tag="uT")
            h1T_ps = psH.tile([128, KT2, OUTF], F32, tag="h1T")
            h_sb = sbt.tile([128, KT2, N], F16, tag="h")
            g2_ps = psO.tile([N, OUTF], F32, tag="g2")
            edf = sbsm.tile([N, HEADS], F32, tag="edf")

            for lo, hi, gf in ((0, HA, gfA), (HA, HEADS, gfB)):
                nh = hi - lo
                cols = (3 * GW if gf is gfA else GW)
                # e_dst slice -> fp32 -> transpose -> [nh, 46]
                nc.vector.tensor_copy(
                    edf[:, lo:hi], gf[:, cols + nh:cols + 2 * nh]
                )
                edT_ps = psT.tile([HEADS, N], F32, tag="tp")
                nc.tensor.transpose(edT_ps[0:nh, :], edf[:, lo:hi], ident_sb[:])
                edT = sbsm.tile([HEADS, N], F32, tag="edT", name=f"edT{lo}")
                nc.vector.tensor_copy(edT[0:nh, :], edT_ps[0:nh, :])
                for h in range(lo, hi):
                    nc.tensor.matmul(
                        ebc_ps[:, h, :],
                        sel_sb[0:nh, N * (h - lo):N * (h - lo + 1)],
                        edT[0:nh, :],
                        start=True,
                        stop=True,
                    )
                # e = leaky_relu(esrc_i + edst_j); u = exp(e) * adj; softmax
                esrc = gf[:, cols:cols + nh]
                nc.vector.tensor_add(
                    e_sb[:, lo:hi, :],
                    ebc_ps[:, lo:hi, :],
                    esrc.unsqueeze(2).broadcast_to([N, nh, N]),
                )
                # leaky_relu in one op: max(e, 0.2*e)
                nc.vector.scalar_tensor_tensor(
                    t02[:, lo:hi, :], e_sb[:, lo:hi, :], 0.2,
                    e_sb[:, lo:hi, :], op0=OP.mult, op1=OP.max,
                )
                # additive adjacency mask (-3e4 -> exp underflows to 0)
                nc.vector.tensor_add(
                    e_sb[:, lo:hi, :],
                    t02[:, lo:hi, :],
                    adj01_sb[:].unsqueeze(1).broadcast_to([N, nh, N]),
                )
                # exp + per-head masked row-sum in one instruction
                for h in range(lo, hi):
                    nc.scalar.activation(
                        u_sb[:, h, :], e_sb[:, h, :], ACTF.Exp,
                        accum_out=s_sb[:, h:h + 1],
                    )
                nc.vector.reciprocal(r_sb[:, lo:hi], s_sb[:, lo:hi])
                nc.vector.tensor_mul(
                    u_sb[:, lo:hi, :],
                    u_sb[:, lo:hi, :],
                    r_sb[:, lo:hi].unsqueeze(2).broadcast_to([N, nh, N]),
                )
                # h1T blocks + ELU + layer-2 partial accumulation
                for h in range(lo, hi):
                    uT_ps = psT.tile([N, N], F32, tag="tp")
                    nc.tensor.transpose(uT_ps[:], u_sb[:, h, :], ident_sb[:])
                    nc.vector.tensor_copy(uT[:, h, :], uT_ps[:])
                    for b in range(2):
                        blk = 2 * h + b
                        nc.tensor.matmul(
                            h1T_ps[:, blk, 0:N],
                            gf[:, F1 * (h - lo) + 128 * b:
                               F1 * (h - lo) + 128 * (b + 1)],
                            uT[:, h, :],
                            start=True,
                            stop=True,
                        )
                blo, bhi = 2 * lo, 2 * hi
                tneg = sbt.tile([128, KT2, N], F32, tag="tneg")
                nc.vector.tensor_scalar_min(
                    tneg[:, blo:bhi, :], h1T_ps[:, blo:bhi, 0:N], 0.0
                )
                texp = sbt.tile([128, KT2, N], F32, tag="texp")
                nc.scalar.activation(
                    texp[:, blo:bhi, :], tneg[:, blo:bhi, :], ACTF.Exp
                )
                tpos = sbt.tile([128, KT2, N], F32, tag="tpos")
                nc.vector.tensor_scalar_max(
                    tpos[:, blo:bhi, :], h1T_ps[:, blo:bhi, 0:N], 0.0
                )
                nc.vector.scalar_tensor_tensor(
                    h_sb[:, blo:bhi, :],
                    texp[:, blo:bhi, :],
                    -1.0,
                    tpos[:, blo:bhi, :],
                    op0=OP.add,
                    op1=OP.add,
                )
                for k in range(blo, bhi):
                    nc.tensor.matmul(
                        g2_ps[:],
                        h_sb[:, k, :],
                        w2_sb[:, k, :],
                        start=(k == 0),
                        stop=(k == KT2 - 1),
                    )

            # ---- layer-2 attention (1 head) + MLP ----
            g2_sb = sbsm.tile([N, OUTF], F32, tag="g2")
            nc.vector.tensor_copy(g2_sb[:], g2_ps[:])
            g2b = sbsm.tile([N, OUTF], F16, tag="g2b")
            nc.vector.tensor_copy(g2b[:], g2_ps[:])

            t2 = sbsm.tile([N, OUTF], F32, tag="t2")
            nc.vector.tensor_mul(t2[:], g2_sb[:], a2sb[:])
            e2s = sbsm.tile([N, 1], F32, tag="e2s")
            nc.vector.tensor_reduce(e2s[:], t2[:], axis=AX.X, op=OP.add)
            nc.vector.tensor_mul(t2[:], g2_sb[:], a2db[:])
            e2d = sbsm.tile([N, 1], F32, tag="e2d")
            nc.vector.tensor_reduce(e2d[:], t2[:], axis=AX.X, op=OP.add)

            e2dT_ps = psS.tile([1, N], F32, tag="er")
            nc.tensor.transpose(e2dT_ps[:], e2d[:], ident_sb[:])
            e2dT = sbsm.tile([1, N], F32, tag="e2dT")
            nc.vector.tensor_copy(e2dT[:], e2dT_ps[:])
            ebc2_ps = psS.tile([N, N], F32, tag="er")
            nc.tensor.matmul(
                ebc2_ps[:], ones46_sb[:], e2dT[:], start=True, stop=True
            )
            e2_sb = sbsm.tile([N, N], F32, tag="e2")
            nc.vector.tensor_scalar(
                e2_sb[:], ebc2_ps[:], e2s[:, 0:1], None, OP.add
            )
            t22 = sbsm.tile([N, N], F32, tag="t22")
            nc.vector.tensor_scalar_mul(t22[:], e2_sb[:], 0.2)
            nc.vector.tensor_max(e2_sb[:], e2_sb[:], t22[:])
            u2_sb = sbsm.tile([N, N], F32, tag="u2")
            nc.scalar.activation(u2_sb[:], e2_sb[:], ACTF.Exp)
            nc.vector.tensor_mul(u2_sb[:], u2_sb[:], adj01_sb[:])
            s2_sb = sbsm.tile([N, 1], F32, tag="s2")
            nc.vector.tensor_reduce(s2_sb[:], u2_sb[:], axis=AX.X, op=OP.add)
            r2_sb = sbsm.tile([N, 1], F32, tag="r2")
            nc.vector.reciprocal(r2_sb[:], s2_sb[:])
            nc.vector.tensor_scalar(
                u2_sb[:], u2_sb[:], r2_sb[:, 0:1], None, OP.mult
            )
            u2T_ps = psT.tile([N, N], F32, tag="tp")
            nc.tensor.transpose(u2T_ps[:], u2_sb[:], ident_sb[:])
            u2T = sbsm.tile([N, N], F16, tag="u2T")
            nc.vector.tensor_copy(u2T[:], u2T_ps[:])
            o2_ps = psO.tile([N, OUTF], F32, tag="o2")
            nc.tensor.matmul(o2_ps[:], u2T[:], g2b[:], start=True, stop=True)
            # mean over the 64 features folded into host-prescaled mw1 (/64)
            m_sb = sbsm.tile([N, 1], F32, tag="m")
            nc.vector.tensor_reduce(m_sb[:], o2_ps[:], axis=AX.X, op=OP.add)

            z1_ps = psS.tile([1, 12], F32, tag="er")
            nc.tensor.matmul(z1_ps[:], m_sb[:], mw1_sb[:], start=True, stop=True)
            z1_sb = sbsm.tile([1, 12], F32, tag="z1")
            nc.vector.tensor_add(z1_sb[:], z1_ps[:], mb1_sb[:])
            zt_sb = sbsm.tile([1, 12], F32, tag="zt")
            nc.vector.tensor_mul(zt_sb[:], z1_sb[:], mw2t_sb[:])
            z2_sb = sbsm.tile([1, 1], F32, tag="z2")
            nc.vector.tensor_reduce(z2_sb[:], zt_sb[:], axis=AX.X, op=OP.add)
            # sigmoid(z2 + mb2) via exp (avoids a Sigmoid table load)
            zb_sb = sbsm.tile([1, 1], F32, tag="zb")
            nc.vector.tensor_add(zb_sb[:], z2_sb[:], mb2_sb[:])
            zn_sb = sbsm.tile([1, 1], F32, tag="zn")
            nc.vector.tensor_scalar_mul(zn_sb[:], zb_sb[:], -1.0)
            ez_sb = sbsm.tile([1, 1], F32, tag="ez")
            nc.scalar.activation(ez_sb[:], zn_sb[:], ACTF.Exp)
            ez1_sb = sbsm.tile([1, 1], F32, tag="ez1")
            nc.vector.tensor_scalar_add(ez1_sb[:], ez_sb[:], 1.0)
            res_sb = sbsm.tile([1, 1], F32, tag="res")
            nc.vector.reciprocal(res_sb[:], ez1_sb[:])
            nc.sync.dma_start(out.ap(), res_sb[:])

    nc.compile()
    return nc


_NC_CACHE = []


def _get_nc():
    if not _NC_CACHE:
        _NC_CACHE.append(build())
    return _NC_CACHE[0]


def _prep_in_maps(x, adj, W1, a1, W2, a2, mw1, mb1, mw2, mb2):
    adj01 = adj[:, :, 0].astype(np.float32)
    shared = {
        "adj01": np.where(adj[:, :, 0], np.float32(0.0), np.float32(-30000.0)),
        "asrcf": np.ascontiguousarray(
            a1[:, :F1].reshape(1, HID).astype(np.float16)
        ),
        "adstf": np.ascontiguousarray(
            a1[:, F1:].reshape(1, HID).astype(np.float16)
        ),
        "a2sf": np.ascontiguousarray(a2[0, :OUTF].reshape(1, OUTF).astype(np.float16)),
        "a2df": np.ascontiguousarray(a2[0, OUTF:].reshape(1, OUTF).astype(np.float16)),
        "w2r": np.ascontiguousarray(
            W2.reshape(KT2, 128, OUTF).transpose(1, 0, 2).reshape(128, KT2 * OUTF)
        ).astype(np.float16),
        "sel": np.kron(np.eye(HEADS, dtype=np.float32), np.ones((1, N), np.float32)),
        "ones46": np.ones((1, N), np.float32),
        "mw1": np.ascontiguousarray(mw1 / np.float32(OUTF)),
        "mb1": mb1.reshape(1, 12).astype(np.float32),
        "mw2t": np.ascontiguousarray(mw2.reshape(1, 12)),
        "mb2": mb2.reshape(1, 1).astype(np.float32),
        "ident": np.eye(N, dtype=np.float32),
    }
    x8 = (x * np.float32(X_SCALE)).astype(NP_F8)
    W18 = (W1 * np.float32(W_SCALE)).astype(NP_F8)
    in_maps = []
    for c in range(NCORES):
        m = dict(shared)
        # x slice, transposed to [128, kt, 46]
        xc = np.ascontiguousarray(x8[:, KC * c:KC * (c + 1)]).T  # [16384, 46]
        m["xs"] = np.ascontiguousarray(
            xc.reshape(KT, 128, N).transpose(1, 0, 2).reshape(128, KT * N)
        )
        # W1 slice, [128, G, KT, GW] flattened
        w1c = W18[KC * c:KC * (c + 1), :].reshape(KT, 128, G, GW)
        m["w1"] = np.ascontiguousarray(
            w1c.transpose(1, 2, 0, 3).reshape(128, G * KT * GW)
        )
        in_maps.append(m)
    return in_maps


def kernel(**inputs):
    x = np.asarray(inputs["x"], dtype=np.float32)
    adj = np.asarray(inputs["adj_mat"]).astype(bool).reshape(N, N, 1)
    W1 = np.asarray(inputs["W1"], dtype=np.float32)
    a1 = np.asarray(inputs["a1"], dtype=np.float32)
    W2 = np.asarray(inputs["W2"], dtype=np.float32)
    a2 = np.asarray(inputs["a2"], dtype=np.float32)
    mw1 = np.asarray(inputs["mlp_w1"], dtype=np.float32)
    mb1 = np.asarray(inputs["mlp_b1"], dtype=np.float32)
    mw2 = np.asarray(inputs["mlp_w2"], dtype=np.float32)
    mb2 = np.asarray(inputs["mlp_b2"], dtype=np.float32)

    nc = _get_nc()
    in_maps = _prep_in_maps(x, adj, W1, a1, W2, a2, mw1, mb1, mw2, mb2)
    res = run_bass_kernel_spmd(nc, in_maps, core_ids=list(range(NCORES)))
    return res.results[0]["out"].reshape(1).astype(np.float32)
